# revision 6
# baseline (speedup 1.0000x reference)
"""Categorical cross-entropy loss kernel for Trainium2 (8 NeuronCores).

Computes: out = [-sum(input * log(target + 1e-8)) / B] for input/target of
shape [B=262144, C=128] float32.

Strategy (data-parallel, fp8-compressed streaming, rel tol 2e-2):
  - Host quantizes both tensors to fp8_e4m3 scaled by 128 (max 128 < 240
    = TRN FP8_EXP4 max normal; quantization rel err ~7e-4).  HBM traffic
    drops 4x vs f32: 8 MiB/core/pass -> ~23.3 us DMA floor at 358 GB/s.
  - Shard along batch across 8 cores (32768 rows each), viewed as
    [128 partitions, 32768 free] fp8, streamed in chunks.
  - log(target): ScalarE ACT computes Ln(q * 1/128 + 1e-8) -> bf16 on
    most columns (1 elem/lane/cycle, would be 27.3 us/pass alone); a
    fraction F_DVE of columns instead uses a DVE exponent-bit trick
    (u8 bits of fp8 -> bf16, then ln(q)-ln(128) ~= a*u + b with the
    mean mantissa correction mu; rel err ~5e-4) to keep ScalarE under
    the DMA floor.
  - multiply+reduce fused on TensorE: matmul(psum += inp8[:, c:c+128].T
    @ logt[:, c:c+128]) accumulates; diag(psum)[k] = sum_p inp*logt of
    column k, so trace(psum) over all accumulated windows = full dot
    product.  53 ns per 128x128 window -> ~13.6 us/pass, PE otherwise
    idle.  Avoids DVE tensor_reduce (1x mode only, 34 us/pass).
  - Per-core output: [128, 128] f32 psum image; host takes trace in
    f64, scales by -1/(128*B).
"""

import math

import numpy as np
import ml_dtypes

import concourse.bass as bass
import concourse.tile as tile
from concourse import bacc, mybir
from concourse.bass_utils import run_bass_kernel_spmd

B, C = 262144, 128
NCORES = 8
ROWS = B // NCORES          # 32768 rows per core
P = 128                     # SBUF partitions
FREE = ROWS * C // P        # 32768 fp8 elems per partition
EPS = 1e-8
SCALE = 128.0               # fp8 carries SCALE*x; max 128 <= 240 (TRN e4m3)
FP8 = ml_dtypes.float8_e4m3

# bit-trick log constants: for positive fp8e4 with bits u = 8*e + m,
# ln(value) ~= ln2 * (u/8 - 7 + MU); logt = ln(value) - ln(SCALE)
MU = 0.0573
BIT_A = math.log(2.0) / 8.0
BIT_B = math.log(2.0) * (MU - 7.0) - math.log(SCALE)

# body chunks stream at full DMA width; tapered tail shrinks the serial
# ACT/DVE->PE chain after the last byte lands
CH_SCHEDULE = [4096] * 6 + [2048] * 3 + [1024, 512, 256, 128, 128]
assert sum(CH_SCHEDULE) == FREE

# fraction of each chunk's columns whose log runs on DVE (bit trick)
# instead of ScalarE Ln; rounded to multiples of 128 inside build.
F_DVE = 0.25

_NC_CACHE = None


def build_nc(repeat: int = 1, ch_schedule=None, io_bufs: int = 3,
             scratch_bufs: int = 3, compute: str = "full",
             f_dve: float = F_DVE, lean_preamble: bool = True,
             small_out: bool = False) -> bass.Bass:
    if ch_schedule is None:
        ch_schedule = CH_SCHEDULE
    assert sum(ch_schedule) == FREE
    nch = len(ch_schedule)
    offs = [0]
    for c in ch_schedule:
        offs.append(offs[-1] + c)
    max_ch = max(ch_schedule)
    nc = bacc.Bacc("TRN2", target_bir_lowering=False, debug=False,
                   num_devices=NCORES)
    if lean_preamble:
        # Bass.__init__ memsets 4 const APs on gpsimd before the init
        # barrier; nothing here reads them, so drop the serial memsets.
        bb = nc.cur_bb.bb
        bb.instructions = [
            i for i in bb.instructions
            if not (isinstance(i, mybir.InstMemset)
                    and i.outs and "const-" in str(i.outs[0]))
        ]
    inp = nc.dram_tensor("input", [ROWS, C], mybir.dt.float8e4,
                         kind="ExternalInput").ap()
    tgt = nc.dram_tensor("target", [ROWS, C], mybir.dt.float8e4,
                         kind="ExternalInput").ap()
    out_w = 1 if small_out else P
    out = nc.dram_tensor("out", [P, out_w], mybir.dt.float32,
                         kind="ExternalOutput").ap()

    inp_v = inp.rearrange("(p n) c -> p (n c)", p=P)
    tgt_v = tgt.rearrange("(p n) c -> p (n c)", p=P)

    with tile.TileContext(nc) as tc:
        with (
            tc.tile_pool(name="eps", bufs=1) as eps_pool,
            tc.tile_pool(name="io", bufs=io_bufs) as io_pool,
            tc.tile_pool(name="scratch", bufs=scratch_bufs) as scratch_pool,
            tc.tile_pool(name="res", bufs=1) as res_pool,
            tc.tile_pool(name="psum", bufs=1, space="PSUM") as psum_pool,
        ):
            if compute != "none":
                eps_t = eps_pool.tile([P, 1], mybir.dt.float32)
                nc.gpsimd.memset(eps_t[:], EPS)

            acc = None
            if compute in ("full", "mm"):
                acc = psum_pool.tile([P, P], mybir.dt.float32)

            niter = nch * repeat
            last_tt = None
            mm_done = 0
            mm_total = (FREE // 128) * repeat if compute in ("full", "mm") \
                else 0
            for it in range(niter):
                j = it % nch
                ch = ch_schedule[j]
                # target first: log engines only need tgt, so they start
                # while input is still in flight
                tt = io_pool.tile([P, max_ch], mybir.dt.float8e4, tag="tgt")
                nc.sync.dma_start(tt[:, :ch], tgt_v[:, offs[j]:offs[j] + ch])
                ti = io_pool.tile([P, max_ch], mybir.dt.float8e4, tag="inp")
                nc.sync.dma_start(ti[:, :ch], inp_v[:, offs[j]:offs[j] + ch])
                last_tt = tt
                if compute == "none":
                    continue
                # split columns: ScalarE Ln on [:s], DVE bit-log on [s:]
                s = ch - (int(round(ch * f_dve)) // 128) * 128
                logt = scratch_pool.tile([P, max_ch], mybir.dt.bfloat16)
                if compute in ("full", "act", "mm"):
                    if s > 0:
                        nc.scalar.activation(
                            logt[:, :s], tt[:, :s],
                            mybir.ActivationFunctionType.Ln,
                            bias=eps_t[:], scale=1.0 / SCALE)
                    if s < ch:
                        ub = scratch_pool.tile([P, max_ch],
                                               mybir.dt.bfloat16, tag="ub")
                        nc.vector.tensor_copy(
                            ub[:, s:ch], tt[:, s:ch].bitcast(mybir.dt.uint8))
                        nc.vector.tensor_scalar(
                            logt[:, s:ch], ub[:, s:ch], BIT_A, BIT_B,
                            mybir.AluOpType.mult, mybir.AluOpType.add)
                if compute in ("full", "mm"):
                    for c0 in range(0, ch, 128):
                        nc.tensor.matmul(
                            acc[:], ti[:, c0:c0 + 128], logt[:, c0:c0 + 128],
                            start=(mm_done == 0),
                            stop=(mm_done == mm_total - 1))
                        mm_done += 1
            if compute in ("full", "mm"):
                res = res_pool.tile([P, P], mybir.dt.float32)
                nc.scalar.activation(res[:], acc[:],
                                     mybir.ActivationFunctionType.Copy)
                nc.sync.dma_start(out[:], res[:, :out_w])
            else:  # timing probes: output garbage, dep only on last tile
                nc.sync.dma_start(out[:, :1], last_tt[:, :4].bitcast(
                    mybir.dt.float32))
    nc.compile()
    return nc


def quantize(x: np.ndarray) -> np.ndarray:
    return (x * np.float32(SCALE)).astype(FP8)


def shard_inputs(inp: np.ndarray, tgt: np.ndarray) -> list[dict]:
    qi, qt = quantize(inp), quantize(tgt)
    return [
        {
            "input": np.ascontiguousarray(qi[i * ROWS:(i + 1) * ROWS]),
            "target": np.ascontiguousarray(qt[i * ROWS:(i + 1) * ROWS]),
        }
        for i in range(NCORES)
    ]


def combine(results: list[dict]) -> np.ndarray:
    total = 0.0
    for r in results:
        total += float(np.trace(np.asarray(r["out"], dtype=np.float64)))
    return np.array([-total / (SCALE * B)], dtype=np.float32)


def kernel(**inputs: np.ndarray) -> np.ndarray:
    global _NC_CACHE
    inp = np.ascontiguousarray(np.asarray(inputs["input"], dtype=np.float32))
    tgt = np.ascontiguousarray(np.asarray(inputs["target"], dtype=np.float32))
    assert inp.shape == (B, C) and tgt.shape == (B, C)

    if _NC_CACHE is None:
        _NC_CACHE = build_nc()
    nc = _NC_CACHE

    res = run_bass_kernel_spmd(nc, shard_inputs(inp, tgt),
                               list(range(NCORES)))
    return combine(res.results)
